# revision 18
# baseline (speedup 1.0000x reference)
"""GCNConv kernel for 8 Trainium2 NeuronCores (Bass/Tile).

Computes out = segment_sum(edge_val * (x @ W)[edge_col], edge_row) + b
as out = (A @ x) @ W + b  (associativity), with:
  - nodes (rows of output) sharded across 8 cores (12500 each)
  - edges partitioned by destination tile (128 rows), grouped 2 tiles per
    "group" and by source bank -> one dma_gather per (group, bank, dtype);
    edges are sorted by source index within each cell so the gather's
    HBM reads are monotone (row-buffer locality)
  - val-threshold precision split: edges with val < THETA are gathered in
    fp8e4m3 (half the HBM gather bytes) with host-precomputed one-hot
    S8 matrices DMA'd in; edges with val >= THETA stay fp16 with S built
    on-chip by two wide DVE tensor_tensor ops using broadcast APs.
  - z[128 nodes, 256] += S_j.T @ X_block per block on the PE in PSUM
    (fp8 and fp16 matmuls accumulate into the same f32 PSUM tile).
  - epilogue per tile: transpose z, project by W (fp16), add bias, store.

x is split into 4 banks of 25000 rows because dma_gather indices are int16.
"""
import os
from contextlib import ExitStack

import ml_dtypes
import numpy as np

import concourse.bass as bass
import concourse.tile as tile
from concourse import bacc, mybir
from concourse.bass_utils import run_bass_kernel_spmd

P = 128
D = 256
N_NODES = 100000
N_EDGES = 3200000
NC = 8
SH = N_NODES // NC          # 12500 rows per core
NT = (SH + P - 1) // P      # 98 tiles per core
GRP = 2                     # tiles per gather group
NG = NT // GRP              # 49 groups
SUP = 4                     # groups per idx/dv/s8 load
NBANK = 4
BS = N_NODES // NBANK       # 25000 rows per bank (fits int16 index)
NCELL = NG * NBANK * 2 * GRP  # cells in block order (g, k, prec, dt)
THETA = 0.45                # edges with val < THETA take the fp8 path

F8 = mybir.dt.float8e4
F16 = mybir.dt.float16
F32 = mybir.dt.float32
I16 = mybir.dt.int16
NPF8 = ml_dtypes.float8_e4m3

_last_results = None        # BassKernelResults of the most recent run


def _build_structure(edge_row, edge_col, edge_val):
    """Sort/pad edges into per-core 128-edge blocks ordered by
    (group of 2 dest tiles, source bank, precision, dest tile), sorted by
    source index within each cell.  Cell structure (nb_cell) is shared
    across cores (padded to the max) so one SPMD program fits all cores.

    Returns (nb_cell [NCELL] int, per-core dict arrays).
    """
    E = edge_row.shape[0]
    core = edge_row // SH
    r_loc = edge_row - core * SH
    t = r_loc // P
    dloc = (r_loc % P).astype(np.float16)
    g = t // GRP
    dt_ = t - g * GRP
    bank = edge_col // BS
    bidx = (edge_col % BS).astype(np.int16)
    prec = (edge_val >= THETA).astype(np.int64)   # 0: fp8, 1: fp16

    cid = ((g.astype(np.int64) * NBANK + bank) * 2 + prec) * GRP + dt_
    gid = core.astype(np.int64) * NCELL + cid
    # sort by (core, cell, src index) -> monotone HBM reads per gather
    order = np.argsort(gid * (BS + 1) + bidx, kind="stable")
    gid_s = gid[order]

    cnt = np.bincount(gid, minlength=NC * NCELL).reshape(NC, NCELL)
    nb_cell = (cnt.max(axis=0) + P - 1) // P        # [NCELL] blocks
    nb_cell = np.maximum(nb_cell, 1)                # keep structure non-empty
    NBLK = int(nb_cell.sum())
    pad_len = NBLK * P

    # slot offset of each cell within a core's padded edge list
    off_cell = np.zeros(NCELL, np.int64)
    flat_off = np.cumsum(nb_cell * P)
    off_cell[1:] = flat_off[:-1]

    # position of each edge within its (core, cell) run
    grp_start = np.zeros(E, np.int64)
    newgrp = np.ones(E, bool)
    newgrp[1:] = gid_s[1:] != gid_s[:-1]
    starts = np.where(newgrp)[0]
    grp_start[starts] = starts
    grp_start = np.maximum.accumulate(grp_start)
    pos_in_grp = np.arange(E) - grp_start

    cid_of_edge = gid_s % NCELL
    core_of_edge = gid_s // NCELL
    dest = off_cell[cid_of_edge] + pos_in_grp

    # per-block precision mask (block-level, shared across cores)
    cell_is8 = (np.arange(NCELL) // GRP) % 2 == 0
    blk_is8 = np.repeat(cell_is8, nb_cell)
    NB8 = int(blk_is8.sum())

    cores = []
    ev16 = edge_val.astype(np.float16)
    for c in range(NC):
        m = core_of_edge == c
        e_ids = order[m]
        d = dest[m]
        idx_arr = np.zeros(pad_len, np.int16)
        dloc_arr = np.zeros(pad_len, np.float16)
        val_arr = np.zeros(pad_len, np.float16)
        idx_arr[d] = bidx[e_ids]
        dloc_arr[d] = dloc[e_ids]
        val_arr[d] = ev16[e_ids]

        # packed gather indices: [128, 8*NBLK] int16 (16-wrap, replicated x8)
        idxp = np.tile(np.ascontiguousarray(idx_arr.reshape(-1, 16).T), (8, 1))
        dl = dloc_arr.reshape(NBLK, P)
        vv = val_arr.reshape(NBLK, P)
        # fp16 blocks -> per-block [dloc, val]: [128, 2*NB16] f16
        dl16 = np.ascontiguousarray(dl[~blk_is8].T)
        vv16 = np.ascontiguousarray(vv[~blk_is8].T)
        nb16 = dl16.shape[1]
        dv = np.empty((P, 2 * nb16), np.float16)
        dv[:, 0::2] = dl16
        dv[:, 1::2] = vv16
        # fp8 blocks -> dense one-hot S8: [128, NB8*128] f8e4m3
        dl8 = dl[blk_is8].astype(np.int64)      # [NB8, 128e]
        vv8 = vv[blk_is8]                       # [NB8, 128e]
        s8m = np.zeros((NB8, P, P), np.float16)  # [j, e, d]
        np.put_along_axis(s8m, dl8[:, :, None], vv8[:, :, None], axis=2)
        s8 = np.ascontiguousarray(
            s8m.transpose(1, 0, 2).reshape(P, NB8 * P)).astype(NPF8)
        cores.append(dict(idxp=idxp, dv=dv, s8=s8))

    return nb_cell, cores


def _build_program(nb_cell):
    """Build the SPMD Bass program for the given cell structure."""
    cells = np.asarray(nb_cell).reshape(NG, NBANK, 2, GRP)
    nb_g = cells.sum(axis=(1, 2, 3))                # [NG] blocks per group
    nb_g8 = cells[:, :, 0, :].sum(axis=(1, 2))      # [NG] fp8 blocks
    nb_g16 = cells[:, :, 1, :].sum(axis=(1, 2))     # [NG] fp16 blocks
    NBLK = int(nb_g.sum())
    NB8 = int(nb_g8.sum())
    NB16 = int(nb_g16.sum())
    g8_max = int(nb_g8.max())
    g16_max = int(nb_g16.max())
    sn_all = [int(nb_g[s:s + SUP].sum()) for s in range(0, NG, SUP)]
    sn8 = [int(nb_g8[s:s + SUP].sum()) for s in range(0, NG, SUP)]
    sn16 = [int(nb_g16[s:s + SUP].sum()) for s in range(0, NG, SUP)]
    out_rows = NT * P

    nc = bacc.Bacc("TRN2", target_bir_lowering=False, debug=False,
                   num_devices=NC, num_swdge_queues=4,
                   dynamic_dma_scratch_size=32768)
    xb_aps = [nc.dram_tensor(f"xb{k}", [BS, D], F16,
                             kind="ExternalInput").ap() for k in range(NBANK)]
    x8_aps = [nc.dram_tensor(f"x8b{k}", [BS, D], F8,
                             kind="ExternalInput").ap() for k in range(NBANK)]
    idxp_ap = nc.dram_tensor("idxp", [P, 8 * NBLK], I16,
                             kind="ExternalInput").ap()
    dv_ap = nc.dram_tensor("dv", [P, 2 * NB16], F16,
                           kind="ExternalInput").ap()
    s8_ap = nc.dram_tensor("s8", [P, NB8 * P], F8,
                           kind="ExternalInput").ap()
    w_ap = nc.dram_tensor("w", [D, D], F16, kind="ExternalInput").ap()
    bias_ap = nc.dram_tensor("bias", [P, D], F32, kind="ExternalInput").ap()
    iota_ap = nc.dram_tensor("iota", [P, P], F16, kind="ExternalInput").ap()
    ident_ap = nc.dram_tensor("ident", [P, P], F16, kind="ExternalInput").ap()
    out_ap = nc.dram_tensor("out", [out_rows, D], F32,
                            kind="ExternalOutput").ap()

    sa_max, s8_max, s16_max = max(sn_all), max(sn8), max(sn16)

    with tile.TileContext(nc) as tc:
        with ExitStack() as ctx:
            const = ctx.enter_context(tc.tile_pool(name="const", bufs=1))
            idxpool = ctx.enter_context(tc.tile_pool(name="idxp", bufs=2))
            dvpool = ctx.enter_context(tc.tile_pool(name="dvp", bufs=2))
            s8pool = ctx.enter_context(tc.tile_pool(name="s8p", bufs=2))
            xg8pool = ctx.enter_context(tc.tile_pool(name="xg8", bufs=3))
            xg16pool = ctx.enter_context(tc.tile_pool(name="xg16", bufs=3))
            swpool = ctx.enter_context(tc.tile_pool(name="swp", bufs=2))
            epool = ctx.enter_context(tc.tile_pool(name="ep", bufs=2))
            zpsum = ctx.enter_context(
                tc.tile_pool(name="zps", bufs=4, space="PSUM"))
            tpsum = ctx.enter_context(
                tc.tile_pool(name="tps", bufs=2, space="PSUM"))
            opsum = ctx.enter_context(
                tc.tile_pool(name="ops", bufs=2, space="PSUM"))

            iota_t = const.tile([P, P], F16, tag="iota")
            nc.sync.dma_start(iota_t[:], iota_ap[:])
            ident_t = const.tile([P, P], F16, tag="ident")
            nc.sync.dma_start(ident_t[:], ident_ap[:])
            w_t = const.tile([P, 2, D], F16, tag="w")
            nc.sync.dma_start(w_t[:], w_ap[:].rearrange("(c k) d -> k c d",
                                                        k=P))
            bias_t = const.tile([P, D], F32, tag="bias")
            nc.sync.dma_start(bias_t[:], bias_ap[:])

            bo = bo8 = bo16 = 0          # global block offsets (all/f8/f16)
            sbo = sbo8 = sbo16 = 0       # offsets at current super start
            idx_t = dv_t = s8_t = None
            for g in range(NG):
                if g % SUP == 0:
                    s = g // SUP
                    sbo, sbo8, sbo16 = bo, bo8, bo16
                    idx_t = idxpool.tile([P, 8 * sa_max], I16, tag="idx")
                    nc.sync.dma_start(
                        idx_t[:, :8 * sn_all[s]],
                        idxp_ap[:, 8 * bo:8 * (bo + sn_all[s])])
                    dv_t = dvpool.tile([P, s16_max, 2], F16, tag="dv")
                    nc.sync.dma_start(
                        dv_t[:, :sn16[s], :],
                        dv_ap[:, 2 * bo16:2 * (bo16 + sn16[s])].rearrange(
                            "p (n two) -> p n two", two=2))
                    s8_t = s8pool.tile([P, s8_max, P], F8, tag="s8")
                    nc.sync.dma_start(
                        s8_t[:, :sn8[s], :],
                        s8_ap[:, P * bo8:P * (bo8 + sn8[s])].rearrange(
                            "p (n q) -> p n q", q=P))
                lo = bo - sbo
                lo8 = bo8 - sbo8
                lo16 = bo16 - sbo16
                gnb8 = int(nb_g8[g])
                gnb16 = int(nb_g16[g])

                xg8 = xg8pool.tile([P, g8_max, D], F8, tag="xg8")
                xg16 = xg16pool.tile([P, g16_max, D], F16, tag="xg16")
                ok = ok8 = ok16 = 0
                okk8 = []
                okk16 = []
                for k in range(NBANK):
                    nb8k = int(cells[g, k, 0, :].sum())
                    nb16k = int(cells[g, k, 1, :].sum())
                    okk8.append(ok8)
                    okk16.append(ok16)
                    n = nb8k * P
                    nc.gpsimd.dma_gather(
                        out_ap=xg8[:, ok8:ok8 + nb8k, :],
                        in_ap=x8_aps[k][:],
                        idxs_ap=idx_t[:, 8 * (lo + ok):8 * (lo + ok + nb8k)],
                        num_idxs=n,
                        num_idxs_reg=n,
                        elem_size=D,
                        single_packet=(n <= 992),
                        queue_num=k,
                    )
                    ok += nb8k
                    ok8 += nb8k
                    n = nb16k * P
                    nc.gpsimd.dma_gather(
                        out_ap=xg16[:, ok16:ok16 + nb16k, :],
                        in_ap=xb_aps[k][:],
                        idxs_ap=idx_t[:, 8 * (lo + ok):8 * (lo + ok + nb16k)],
                        num_idxs=n,
                        num_idxs_reg=n,
                        elem_size=D,
                        single_packet=(n <= 992),
                        queue_num=k,
                    )
                    ok += nb16k
                    ok16 += nb16k

                sw = swpool.tile([P, g16_max, P], F16, tag="sw")
                dloc_b = dv_t[:, lo16:lo16 + gnb16, 0:1].broadcast_to(
                    (P, gnb16, P))
                val_b = dv_t[:, lo16:lo16 + gnb16, 1:2].broadcast_to(
                    (P, gnb16, P))
                iota_b = iota_t[:].unsqueeze(1).broadcast_to((P, gnb16, P))
                nc.vector.tensor_tensor(out=sw[:, :gnb16, :], in0=iota_b,
                                        in1=dloc_b,
                                        op=mybir.AluOpType.is_equal)
                nc.vector.tensor_tensor(out=sw[:, :gnb16, :],
                                        in0=sw[:, :gnb16, :],
                                        in1=val_b, op=mybir.AluOpType.mult)

                for dt_i in range(GRP):
                    t = g * GRP + dt_i
                    bl8 = []
                    bl16 = []
                    for k in range(NBANK):
                        seg = okk8[k] + (int(cells[g, k, 0, 0]) if dt_i else 0)
                        bl8.extend(range(seg, seg + int(cells[g, k, 0, dt_i])))
                        seg = okk16[k] + (int(cells[g, k, 1, 0]) if dt_i else 0)
                        bl16.extend(range(seg,
                                          seg + int(cells[g, k, 1, dt_i])))
                    nmm = len(bl8) + len(bl16)
                    z_ps = zpsum.tile([P, D], F32, tag="zps")
                    i = 0
                    for jj in bl8:
                        nc.tensor.matmul(out=z_ps[:],
                                         lhsT=s8_t[:, lo8 + jj, :],
                                         rhs=xg8[:, jj, :],
                                         start=(i == 0), stop=(i == nmm - 1))
                        i += 1
                    for jj in bl16:
                        nc.tensor.matmul(out=z_ps[:], lhsT=sw[:, jj, :],
                                         rhs=xg16[:, jj, :],
                                         start=(i == 0), stop=(i == nmm - 1))
                        i += 1

                    z_sb = epool.tile([P, D], F16, tag="zsb")
                    nc.scalar.copy(z_sb[:], z_ps[:])
                    o_ps = opsum.tile([P, D], F32, tag="ops")
                    for ch in range(2):
                        zt_ps = tpsum.tile([P, P], F16, tag="ztps")
                        nc.tensor.transpose(zt_ps[:],
                                            z_sb[:, ch * P:(ch + 1) * P],
                                            ident_t[:])
                        zt_sb = epool.tile([P, P], F16, tag="ztsb")
                        nc.scalar.copy(zt_sb[:], zt_ps[:])
                        nc.tensor.matmul(out=o_ps[:], lhsT=zt_sb[:],
                                         rhs=w_t[:, ch, :],
                                         start=(ch == 0), stop=(ch == 1))
                    o_sb = epool.tile([P, D], F32, tag="osb")
                    nc.vector.tensor_add(o_sb[:], o_ps[:], bias_t[:])
                    nc.sync.dma_start(out_ap[t * P:(t + 1) * P, :], o_sb[:])
                bo += int(nb_g[g])
                bo8 += gnb8
                bo16 += gnb16
    nc.compile()
    return nc


def kernel(x, edge_row, edge_col, edge_val, weight, b):
    global _last_results
    assert x.shape == (N_NODES, D)

    nb_cell, cores = _build_structure(
        np.asarray(edge_row), np.asarray(edge_col), np.asarray(edge_val))
    nc = _build_program(nb_cell)

    x16 = np.asarray(x, np.float32).astype(np.float16)
    x8 = x16.astype(NPF8)
    banks = [np.ascontiguousarray(x16[k * BS:(k + 1) * BS])
             for k in range(NBANK)]
    banks8 = [np.ascontiguousarray(x8[k * BS:(k + 1) * BS])
              for k in range(NBANK)]
    w16 = np.asarray(weight, np.float32).astype(np.float16)
    bias = np.broadcast_to(
        np.asarray(b, np.float32)[None, :], (P, D)).copy()
    iota = np.tile(np.arange(P, dtype=np.float16)[None, :], (P, 1))
    ident = np.eye(P, dtype=np.float16)

    in_maps = []
    for c in range(NC):
        m = {f"xb{k}": banks[k] for k in range(NBANK)}
        m.update({f"x8b{k}": banks8[k] for k in range(NBANK)})
        m.update(idxp=cores[c]["idxp"], dv=cores[c]["dv"], s8=cores[c]["s8"],
                 w=w16, bias=bias, iota=iota, ident=ident)
        in_maps.append(m)

    trace = bool(os.environ.get("KERNEL_TRACE"))
    res = run_bass_kernel_spmd(nc, in_maps, list(range(NC)), trace=trace)
    _last_results = res

    out = np.concatenate([res.results[c]["out"][:SH] for c in range(NC)],
                         axis=0)
    return out.astype(np.float32)
